# revision 3
# baseline (speedup 1.0000x reference)
"""Density-aware Chamfer distance kernel for Trainium2 (8 NeuronCores).

Problem: x,y [4, 8192, 3] f32. Needs, per batch: row-min + argmin of the
8192x8192 pairwise squared-distance matrix in both directions, density
counts, then a scalar loss.

Device strategy (SPMD, 8 cores = 4 batches x 2 directions): each core
computes one full "queries vs candidates" nearest-neighbor job:
  core 2b  : queries=x[b], candidates=y[b]  -> dist1/idx1
  core 2b+1: queries=y[b], candidates=x[b]  -> dist2/idx2
The PE computes s = 2*q.c - qq - cc = -(squared distance) via a K=5
matmul (coords*2, -qq, -1 rows against coords, 1, cc), ScalarE copies
PSUM->SBUF, VectorE max/max_index reduce each 128-query row chunk to the
top-1 value+index (max of negated distance = min distance, first-index
tie-break matches jnp.argmin). The O(N) tail (bincount, weights, loss)
runs on host.
"""

import numpy as np

import concourse.bacc as bacc
import concourse.mybir as mybir
import concourse.tile as tile
from concourse.bass_utils import run_bass_kernel_spmd

B = 4
N = 8192  # x points per batch (queries of job 2b)
M = 8192  # y points per batch
P = 128  # partitions
RCH = N // P  # row chunks per job
TW = 512  # matmul moving-operand width (one PSUM bank of f32)
CPW = 2048  # psum->sbuf copy width (4 banks)
NCP = M // CPW
ALPHA = 1000.0
EPS = 1e-6

TRACE = False
TRACE_KW = {}
LAST_RESULTS = None  # BassKernelResults of the most recent run (for test.py)

_CACHE = {}


def _build():
    nc = bacc.Bacc("TRN2", target_bir_lowering=False)
    f32 = mybir.dt.float32
    lhsT = nc.dram_tensor("lhsT", [5, N], f32, kind="ExternalInput")
    rhs = nc.dram_tensor("rhs", [5, M], f32, kind="ExternalInput")
    outv = nc.dram_tensor("outv", [P, RCH * 8], f32, kind="ExternalOutput")
    outi = nc.dram_tensor("outi", [P, RCH * 8], mybir.dt.uint32, kind="ExternalOutput")

    with tile.TileContext(nc) as tc:
        with (
            tc.tile_pool(name="const", bufs=1) as cpool,
            tc.tile_pool(name="rowbuf", bufs=2) as rpool,
            tc.tile_pool(name="psum", bufs=2, space="PSUM") as ppool,
        ):
            lhsT_sb = cpool.tile([5, N], f32)
            nc.sync.dma_start(lhsT_sb[:], lhsT.ap())
            rhs_sb = cpool.tile([5, M], f32)
            nc.sync.dma_start(rhs_sb[:], rhs.ap())
            outv_sb = cpool.tile([P, RCH * 8], f32)
            outi_sb = cpool.tile([P, RCH * 8], mybir.dt.uint32)

            for r in range(RCH):
                rowbuf = rpool.tile([P, M], f32)
                stat = lhsT_sb[:, r * P : (r + 1) * P]
                for c in range(NCP):
                    ps = ppool.tile([P, CPW], f32)
                    for t in range(CPW // TW):
                        col = c * CPW + t * TW
                        nc.tensor.matmul(
                            ps[:, t * TW : (t + 1) * TW],
                            stat,
                            rhs_sb[:, col : col + TW],
                            start=True,
                            stop=True,
                        )
                    nc.scalar.copy(rowbuf[:, c * CPW : (c + 1) * CPW], ps[:])
                vs = outv_sb[:, r * 8 : (r + 1) * 8]
                nc.vector.max(out=vs, in_=rowbuf[:])
                nc.vector.max_index(
                    out=outi_sb[:, r * 8 : (r + 1) * 8], in_max=vs, in_values=rowbuf[:]
                )
            nc.sync.dma_start(outv.ap(), outv_sb[:])
            nc.sync.dma_start(outi.ap(), outi_sb[:])
    nc.compile()
    return nc


def _prep_job(q, c):
    """q: queries [n,3] f32, c: candidates [m,3] f32 -> in_map."""
    qq = np.sum(q * q, axis=1)
    cc = np.sum(c * c, axis=1)
    lhsT = np.empty((5, q.shape[0]), np.float32)
    lhsT[0:3] = (2.0 * q).T
    lhsT[3] = -qq
    lhsT[4] = -1.0
    rhs = np.empty((5, c.shape[0]), np.float32)
    rhs[0:3] = c.T
    rhs[3] = 1.0
    rhs[4] = cc
    return {"lhsT": lhsT, "rhs": rhs}


def _decode(res_map):
    """Per-core outputs -> (min_dist [n], argmin [n])."""
    vals = res_map["outv"][:, ::8].T.reshape(-1)  # query q = r*128+p
    idxs = res_map["outi"][:, ::8].T.reshape(-1).astype(np.int64)
    return -vals.astype(np.float64), idxs


def kernel(x, y):
    global LAST_RESULTS
    x = np.ascontiguousarray(x, dtype=np.float32)
    y = np.ascontiguousarray(y, dtype=np.float32)

    in_maps = []
    for b in range(B):
        in_maps.append(_prep_job(x[b], y[b]))
        in_maps.append(_prep_job(y[b], x[b]))

    if "nc" not in _CACHE:
        _CACHE["nc"] = _build()
    res = run_bass_kernel_spmd(
        _CACHE["nc"], in_maps, core_ids=list(range(8)), trace=TRACE, **TRACE_KW
    )
    LAST_RESULTS = res

    total = 0.0
    for b in range(B):
        dist1, idx1 = _decode(res.results[2 * b])
        dist2, idx2 = _decode(res.results[2 * b + 1])
        count1 = np.bincount(idx1, minlength=M).astype(np.float64)
        count2 = np.bincount(idx2, minlength=N).astype(np.float64)
        w1 = 1.0 / (count1[idx1] + EPS)
        w2 = 1.0 / (count2[idx2] + EPS)
        loss1 = np.mean(1.0 - np.exp(-dist1 * ALPHA) * w1)
        loss2 = np.mean(1.0 - np.exp(-dist2 * ALPHA) * w2)
        total += (loss1 + loss2) / 2.0
    return np.array(total / B, dtype=np.float32)


# revision 7
# speedup vs baseline: 1.5795x; 1.5795x over previous
"""Density-aware Chamfer distance kernel for Trainium2 (8 NeuronCores).

Problem: x,y [4, 8192, 3] f32. Needs, per batch: row-min + argmin of the
8192x8192 pairwise squared-distance matrix in both directions, density
counts, then a scalar loss.

Device strategy (SPMD, 8 cores = 4 batches x 2 directions): each core
computes one full "queries vs candidates" nearest-neighbor job:
  core 2b  : queries=x[b], candidates=y[b]  -> dist1/idx1
  core 2b+1: queries=y[b], candidates=x[b]  -> dist2/idx2
The PE computes s = 2*q.c - qq - cc = -(squared distance) via a K=5
matmul (coords*2, -qq, -1 rows against coords, 1, cc), ScalarE copies
PSUM->SBUF, VectorE max/max_index reduce each 128-query row chunk to the
top-1 value+index (max of negated distance = min distance, first-index
tie-break matches jnp.argmin). The O(N) tail (bincount, weights, loss)
runs on host.
"""

import ml_dtypes
import numpy as np

import concourse.bacc as bacc
import concourse.mybir as mybir
import concourse.tile as tile
from concourse.bass_utils import run_bass_kernel_spmd

BF16 = ml_dtypes.bfloat16

B = 4
N = 8192  # x points per batch (queries of job 2b)
M = 8192  # y points per batch
P = 128  # partitions
RCH = N // P  # row chunks per job
TW = 512  # matmul moving-operand width (one PSUM bank of f32)
CPW = 2048  # psum->sbuf copy width (4 banks)
NCP = M // CPW
ALPHA = 1000.0
EPS = 1e-6

TRACE = False
TRACE_KW = {}
LAST_RESULTS = None  # BassKernelResults of the most recent run (for test.py)

_CACHE = {}


K = 24  # contraction rows of the split-bf16 distance matmul


def _build():
    nc = bacc.Bacc("TRN2", target_bir_lowering=False)
    f32 = mybir.dt.float32
    bf16 = mybir.dt.bfloat16
    lhsT = nc.dram_tensor("lhsT", [K, N], bf16, kind="ExternalInput")
    rhs = nc.dram_tensor("rhs", [K, M], bf16, kind="ExternalInput")
    outv = nc.dram_tensor("outv", [P, RCH * 8], f32, kind="ExternalOutput")
    outi = nc.dram_tensor("outi", [P, RCH * 8], mybir.dt.uint32, kind="ExternalOutput")

    with tile.TileContext(nc) as tc:
        with (
            tc.tile_pool(name="const", bufs=1) as cpool,
            tc.tile_pool(name="rowbuf", bufs=2) as rpool,
            tc.tile_pool(name="psum", bufs=2, space="PSUM") as ppool,
        ):
            lhsT_sb = cpool.tile([K, N], bf16)
            nc.sync.dma_start(lhsT_sb[:], lhsT.ap())
            rhs_sb = cpool.tile([K, M], bf16)
            nc.sync.dma_start(rhs_sb[:], rhs.ap())
            outv_sb = cpool.tile([P, RCH * 8], f32)
            outi_sb = cpool.tile([P, RCH * 8], mybir.dt.uint32)

            for r in range(RCH):
                rowbuf = rpool.tile([P, M], f32)
                stat = lhsT_sb[:, r * P : (r + 1) * P]
                for c in range(NCP):
                    ps = ppool.tile([P, CPW], f32)
                    for t in range(CPW // TW):
                        col = c * CPW + t * TW
                        nc.tensor.matmul(
                            ps[:, t * TW : (t + 1) * TW],
                            stat,
                            rhs_sb[:, col : col + TW],
                            start=True,
                            stop=True,
                        )
                    nc.scalar.copy(rowbuf[:, c * CPW : (c + 1) * CPW], ps[:])
                vs = outv_sb[:, r * 8 : (r + 1) * 8]
                nc.vector.max(out=vs, in_=rowbuf[:])
                nc.vector.max_index(
                    out=outi_sb[:, r * 8 : (r + 1) * 8], in_max=vs, in_values=rowbuf[:]
                )
            nc.sync.dma_start(outv.ap(), outv_sb[:])
            nc.sync.dma_start(outi.ap(), outi_sb[:])
    nc.compile()
    return nc


def _split3(v):
    """fp32 -> three bf16 arrays whose sum reproduces v to ~2^-27 rel."""
    h = v.astype(BF16)
    r = v - h.astype(np.float32)
    m = r.astype(BF16)
    l = (r - m.astype(np.float32)).astype(BF16)
    return h, m, l


def _prep_job(q, c):
    """q: queries [n,3] f32, c: candidates [m,3] f32 -> in_map.

    Builds the K=24-row split-bf16 factors of s = 2*q.c - qq - cc.
    Per coordinate the six significant split products (hh, hm, mh, hl,
    lh, mm) are kept; dropped terms are O(2^-27).
    """
    n, m_ = q.shape[0], c.shape[0]
    lhsT = np.zeros((K, n), BF16)
    rhs = np.zeros((K, m_), BF16)
    row = 0
    for k in range(3):
        ah, am, al = _split3(2.0 * q[:, k])
        bh, bm, bl = _split3(c[:, k].copy())
        for a_, b_ in ((ah, bh), (ah, bm), (am, bh), (ah, bl), (al, bh), (am, bm)):
            lhsT[row] = a_
            rhs[row] = b_
            row += 1
    qh, qm, ql = _split3(-np.sum(q * q, axis=1))
    for a_ in (qh, qm, ql):
        lhsT[row] = a_
        rhs[row] = np.ones(m_, BF16)
        row += 1
    ch, cm, cl = _split3(np.sum(c * c, axis=1))
    for b_ in (ch, cm, cl):
        lhsT[row] = np.full(n, -1.0, BF16)
        rhs[row] = b_
        row += 1
    assert row == K
    return {"lhsT": lhsT, "rhs": rhs}


def _decode(res_map):
    """Per-core outputs -> (min_dist [n], argmin [n])."""
    vals = res_map["outv"][:, ::8].T.reshape(-1)  # query q = r*128+p
    idxs = res_map["outi"][:, ::8].T.reshape(-1).astype(np.int64)
    return -vals.astype(np.float64), idxs


def kernel(x, y):
    global LAST_RESULTS
    x = np.ascontiguousarray(x, dtype=np.float32)
    y = np.ascontiguousarray(y, dtype=np.float32)

    in_maps = []
    for b in range(B):
        in_maps.append(_prep_job(x[b], y[b]))
        in_maps.append(_prep_job(y[b], x[b]))

    if "nc" not in _CACHE:
        _CACHE["nc"] = _build()
    res = run_bass_kernel_spmd(
        _CACHE["nc"], in_maps, core_ids=list(range(8)), trace=TRACE, **TRACE_KW
    )
    LAST_RESULTS = res

    total = 0.0
    for b in range(B):
        dist1, idx1 = _decode(res.results[2 * b])
        dist2, idx2 = _decode(res.results[2 * b + 1])
        count1 = np.bincount(idx1, minlength=M).astype(np.float64)
        count2 = np.bincount(idx2, minlength=N).astype(np.float64)
        w1 = 1.0 / (count1[idx1] + EPS)
        w2 = 1.0 / (count2[idx2] + EPS)
        loss1 = np.mean(1.0 - np.exp(-dist1 * ALPHA) * w1)
        loss2 = np.mean(1.0 - np.exp(-dist2 * ALPHA) * w2)
        total += (loss1 + loss2) / 2.0
    return np.array(total / B, dtype=np.float32)


# revision 8
# speedup vs baseline: 11.9588x; 7.5711x over previous
"""Density-aware Chamfer distance kernel for Trainium2 (8 NeuronCores).

Problem: x,y [4, 8192, 3] f32. Needs, per batch: row-min + argmin of the
8192x8192 pairwise squared-distance matrix in both directions, density
counts, then a scalar loss.

Strategy (SPMD, 8 cores = 4 batches x 2 directions); each core runs one
"queries vs candidates" nearest-neighbor job:
  core 2b  : queries=x[b], candidates=y[b]  -> dist1/idx1
  core 2b+1: queries=y[b], candidates=x[b]  -> dist2/idx2

Instead of scanning all 8192 candidates per query (memory/vector-bound),
the host groups queries into 64 spatially-compact blocks of 128 (4x4x4
quantile slabs) and gathers, per block, the candidates inside the block
bbox expanded by MARGIN. A query whose device-found nearest distance
exceeds its guaranteed-coverage radius is recomputed exactly on host
(rare: ~0.4%); correctness never depends on the heuristic.

Device per block: PE computes s = 2*q.c - qq - cc = -(squared distance)
with a K=24 split-bf16 matmul (fp32-accurate, 4x faster than fp32
matmul), ScalarE copies PSUM->SBUF, VectorE max/max_index reduce to the
top-1 value+index per query (max of negated distance = min distance;
first-index tie-break + ascending-gathered candidates match jnp.argmin).
The O(N) tail (bincount, weights, loss) runs on host.
"""

import ml_dtypes
import numpy as np

import concourse.bacc as bacc
import concourse.mybir as mybir
import concourse.tile as tile
from concourse.bass_utils import run_bass_kernel_spmd

BF16 = ml_dtypes.bfloat16

B = 4
N = 8192  # points per cloud
P = 128  # partitions = queries per block
NB = N // P  # 64 blocks
CAND = 768  # candidate slots per block
K = 24  # contraction rows of the split-bf16 distance matmul
MARGIN = 0.0625
ALPHA = 1000.0
EPS = 1e-6

TRACE = False
TRACE_KW = {}
LAST_RESULTS = None  # BassKernelResults of the most recent run (for test.py)

_CACHE = {}


def _build():
    nc = bacc.Bacc("TRN2", target_bir_lowering=False)
    f32 = mybir.dt.float32
    bf16 = mybir.dt.bfloat16
    lhsT = nc.dram_tensor("lhsT", [K, N], bf16, kind="ExternalInput")
    rhs = nc.dram_tensor("rhs", [K, NB * CAND], bf16, kind="ExternalInput")
    outv = nc.dram_tensor("outv", [P, NB * 8], f32, kind="ExternalOutput")
    outi = nc.dram_tensor("outi", [P, NB * 8], mybir.dt.uint32, kind="ExternalOutput")

    with tile.TileContext(nc) as tc:
        with (
            tc.tile_pool(name="const", bufs=1) as cpool,
            tc.tile_pool(name="rowbuf", bufs=4) as rpool,
            tc.tile_pool(name="psum", bufs=4, space="PSUM") as ppool,
        ):
            lhsT_sb = cpool.tile([K, N], bf16)
            nc.sync.dma_start(lhsT_sb[:], lhsT.ap())
            rhs_sb = cpool.tile([K, NB * CAND], bf16)
            nc.sync.dma_start(rhs_sb[:], rhs.ap())
            outv_sb = cpool.tile([P, NB * 8], f32)
            outi_sb = cpool.tile([P, NB * 8], mybir.dt.uint32)

            for r in range(NB):
                rowbuf = rpool.tile([P, CAND], f32)
                stat = lhsT_sb[:, r * P : (r + 1) * P]
                ps = ppool.tile([P, CAND], f32)
                base = r * CAND
                nc.tensor.matmul(
                    ps[:, 0:512], stat, rhs_sb[:, base : base + 512],
                    start=True, stop=True,
                )
                nc.tensor.matmul(
                    ps[:, 512:CAND], stat, rhs_sb[:, base + 512 : base + CAND],
                    start=True, stop=True,
                )
                nc.scalar.copy(rowbuf[:], ps[:])
                vs = outv_sb[:, r * 8 : (r + 1) * 8]
                nc.vector.max(out=vs, in_=rowbuf[:])
                nc.vector.max_index(
                    out=outi_sb[:, r * 8 : (r + 1) * 8], in_max=vs, in_values=rowbuf[:]
                )
            nc.sync.dma_start(outv.ap(), outv_sb[:])
            nc.sync.dma_start(outi.ap(), outi_sb[:])
    nc.compile()
    return nc


def _split3(v):
    """fp32 -> three bf16 arrays whose sum reproduces v to ~2^-27 rel."""
    v = np.asarray(v, np.float32)
    h = v.astype(BF16)
    r = v - h.astype(np.float32)
    m = r.astype(BF16)
    l = (r - m.astype(np.float32)).astype(BF16)
    return h, m, l


def _slab_blocks(pts):
    """4x4x4 quantile partition -> perm [N] s.t. block r = perm[128r:128r+128]."""
    ix = np.argsort(pts[:, 0], kind="stable")
    out = []
    for i in range(4):
        sx = ix[i * 2048 : (i + 1) * 2048]
        iy = sx[np.argsort(pts[sx, 1], kind="stable")]
        for j in range(4):
            sy = iy[j * 512 : (j + 1) * 512]
            iz = sy[np.argsort(pts[sy, 2], kind="stable")]
            out.append(iz)
    return np.concatenate(out)


# per-coordinate split-product row schedule: (query component, cand component)
_ROWS = ((0, 0), (0, 1), (1, 0), (0, 2), (2, 0), (1, 1))


class _Job:
    """Host-side bucketization state for one (queries, candidates) job."""

    def __init__(self, q, c):
        self.q, self.c = q, c
        self.perm = _slab_blocks(q)
        qs = q[self.perm]  # sorted queries, block r = rows 128r:128r+128
        self.qs = qs
        c64 = c.astype(np.float64)

        lo = np.empty((NB, 3)); hi = np.empty((NB, 3)); marg = np.full(NB, MARGIN)
        cand_map = np.zeros((NB, CAND), np.int64)
        counts = np.zeros(NB, np.int64)
        gath = np.zeros((NB, CAND, 3), np.float32)
        for r in range(NB):
            p = qs[r * P : (r + 1) * P].astype(np.float64)
            lo[r], hi[r] = p.min(0), p.max(0)
            m = MARGIN
            for _ in range(30):
                sel = np.nonzero(
                    np.all((c64 >= lo[r] - m) & (c64 <= hi[r] + m), axis=1)
                )[0]
                if len(sel) <= CAND:
                    break
                m *= 0.85
            marg[r] = m
            k = len(sel)
            counts[r] = k
            cand_map[r, :k] = sel
            if k < CAND:
                cand_map[r, k:] = sel[0] if k else 0
            gath[r] = c[cand_map[r]]
        self.lo, self.hi, self.marg = lo, hi, marg
        self.cand_map, self.counts = cand_map, counts

        # lhsT [K, N] from sorted queries; rhs [K, NB*CAND] from gathered cands
        lhsT = np.zeros((K, N), BF16)
        rhs = np.zeros((K, NB * CAND), BF16)
        g = gath.reshape(NB * CAND, 3)
        row = 0
        for k in range(3):
            a = _split3(2.0 * qs[:, k])
            b = _split3(g[:, k])
            for ai, bi in _ROWS:
                lhsT[row] = a[ai]
                rhs[row] = b[bi]
                row += 1
        a = _split3(-np.sum(qs * qs, axis=1))
        for t in range(3):
            lhsT[row] = a[t]
            rhs[row] = np.ones(NB * CAND, BF16)
            row += 1
        b = _split3(np.sum(g * g, axis=1))
        for t in range(3):
            lhsT[row] = np.full(N, -1.0, BF16)
            rhs[row] = b[t]
            row += 1
        assert row == K
        self.in_map = {"lhsT": lhsT, "rhs": rhs}

    def finish(self, res_map):
        """Decode device outputs; exact host fallback where the coverage
        guarantee fails. Returns (dist [N], idx [N]) in original order."""
        vals = res_map["outv"][:, ::8].T.reshape(-1)  # sorted-query order
        slots = res_map["outi"][:, ::8].T.reshape(-1).astype(np.int64)
        d_dev = -vals.astype(np.float64)
        blk = np.arange(N) // P
        idx_dev = self.cand_map[blk, slots]

        qs64 = self.qs.astype(np.float64)
        r_in = np.minimum(
            (qs64 - self.lo[blk]).min(1), (self.hi[blk] - qs64).min(1)
        )
        m_q = self.marg[blk] + np.maximum(r_in, 0.0)
        ok = np.sqrt(np.maximum(d_dev, 0.0)) + 1e-3 <= m_q
        ok &= self.counts[blk] > 0
        bad = np.nonzero(~ok)[0]
        if len(bad):
            qb = self.qs[bad]
            d = (
                np.sum(qb * qb, axis=1, keepdims=True)
                - 2.0 * (qb @ self.c.T)
                + np.sum(self.c * self.c, axis=1)[None, :]
            )
            idx_dev[bad] = np.argmin(d, axis=1)
            d_dev[bad] = d[np.arange(len(bad)), idx_dev[bad]]

        dist = np.empty(N); idx = np.empty(N, np.int64)
        dist[self.perm] = d_dev
        idx[self.perm] = idx_dev
        return dist, idx


def kernel(x, y):
    global LAST_RESULTS
    x = np.ascontiguousarray(x, dtype=np.float32)
    y = np.ascontiguousarray(y, dtype=np.float32)

    jobs = []
    for b in range(B):
        jobs.append(_Job(x[b], y[b]))
        jobs.append(_Job(y[b], x[b]))

    if "nc" not in _CACHE:
        _CACHE["nc"] = _build()
    res = run_bass_kernel_spmd(
        _CACHE["nc"],
        [j.in_map for j in jobs],
        core_ids=list(range(8)),
        trace=TRACE,
        **TRACE_KW,
    )
    LAST_RESULTS = res

    total = 0.0
    for b in range(B):
        dist1, idx1 = jobs[2 * b].finish(res.results[2 * b])
        dist2, idx2 = jobs[2 * b + 1].finish(res.results[2 * b + 1])
        count1 = np.bincount(idx1, minlength=N).astype(np.float64)
        count2 = np.bincount(idx2, minlength=N).astype(np.float64)
        w1 = 1.0 / (count1[idx1] + EPS)
        w2 = 1.0 / (count2[idx2] + EPS)
        loss1 = np.mean(1.0 - np.exp(-dist1 * ALPHA) * w1)
        loss2 = np.mean(1.0 - np.exp(-dist2 * ALPHA) * w2)
        total += (loss1 + loss2) / 2.0
    return np.array(total / B, dtype=np.float32)


# revision 11
# speedup vs baseline: 17.7893x; 1.4875x over previous
"""Density-aware Chamfer distance kernel for Trainium2 (8 NeuronCores).

Problem: x,y [4, 8192, 3] f32. Needs, per batch: row-min + argmin of the
8192x8192 pairwise squared-distance matrix in both directions, density
counts, then a scalar loss.

Strategy (SPMD, 8 cores = 4 batches x 2 directions); each core runs one
"queries vs candidates" nearest-neighbor job:
  core 2b  : queries=x[b], candidates=y[b]  -> dist1/idx1
  core 2b+1: queries=y[b], candidates=x[b]  -> dist2/idx2

Instead of scanning all 8192 candidates per query (memory/vector-bound),
the host groups queries into 64 spatially-compact blocks of 128 (4x4x4
quantile slabs) and gathers, per block, the candidates inside the block
bbox expanded by MARGIN. A query whose device-found nearest distance
exceeds its guaranteed-coverage radius is recomputed exactly on host
(rare: ~0.4%); correctness never depends on the heuristic.

Device per block: PE computes s = 2*q.c - qq - cc = -(squared distance)
with a K=24 split-bf16 matmul (fp32-accurate, 4x faster than fp32
matmul), ScalarE copies PSUM->SBUF, VectorE max/max_index reduce to the
top-1 value+index per query (max of negated distance = min distance;
first-index tie-break + ascending-gathered candidates match jnp.argmin).
The O(N) tail (bincount, weights, loss) runs on host.
"""

import ml_dtypes
import numpy as np

import concourse.bacc as bacc
import concourse.mybir as mybir
import concourse.tile as tile
from concourse.bass_utils import run_bass_kernel_spmd

BF16 = ml_dtypes.bfloat16

B = 4
N = 8192  # points per cloud
P = 128  # partitions = queries per block
NB = N // P  # 64 blocks
CAND = 512  # candidate slots per block
RCHUNK = 8  # rhs DMA streaming: blocks per chunk
OCHUNK = 16  # output DMA: blocks per chunk
K = 24  # contraction rows of the split-bf16 distance matmul
MARGIN = 0.0625
ALPHA = 1000.0
EPS = 1e-6

TRACE = False
TRACE_KW = {}
LAST_RESULTS = None  # BassKernelResults of the most recent run (for test.py)

_CACHE = {}


def _build():
    nc = bacc.Bacc("TRN2", target_bir_lowering=False)
    f32 = mybir.dt.float32
    bf16 = mybir.dt.bfloat16
    lhsT = nc.dram_tensor("lhsT", [K, N], bf16, kind="ExternalInput")
    rhs = nc.dram_tensor("rhs", [K, NB * CAND], bf16, kind="ExternalInput")
    outv = nc.dram_tensor("outv", [P, NB * 8], f32, kind="ExternalOutput")
    outi = nc.dram_tensor("outi", [P, NB * 8], mybir.dt.uint32, kind="ExternalOutput")

    with tile.TileContext(nc) as tc:
        with (
            tc.tile_pool(name="const", bufs=1) as cpool,
            tc.tile_pool(name="rowbuf", bufs=4) as rpool,
            tc.tile_pool(name="psum", bufs=6, space="PSUM") as ppool,
        ):
            lhsT_sb = cpool.tile([K, N], bf16)
            nc.sync.dma_start(lhsT_sb[:], lhsT.ap())
            # stream rhs in chunks (separate tiles so matmuls only wait on
            # the chunk they read, not the whole 1.6MB transfer)
            rhs_sb = []
            for ci in range(NB // RCHUNK):
                t = cpool.tile([K, RCHUNK * CAND], bf16, name=f"rhs{ci}")
                w = RCHUNK * CAND
                nc.sync.dma_start(t[:], rhs.ap()[:, ci * w : (ci + 1) * w])
                rhs_sb.append(t)
            outv_sb = [
                cpool.tile([P, OCHUNK * 8], f32, name=f"ov{ci}")
                for ci in range(NB // OCHUNK)
            ]
            outi_sb = [
                cpool.tile([P, OCHUNK * 8], mybir.dt.uint32, name=f"oi{ci}")
                for ci in range(NB // OCHUNK)
            ]

            for r in range(NB):
                rowbuf = rpool.tile([P, CAND], f32)
                stat = lhsT_sb[:, r * P : (r + 1) * P]
                ps = ppool.tile([P, CAND], f32)
                rsrc = rhs_sb[r // RCHUNK]
                base = (r % RCHUNK) * CAND
                nc.tensor.matmul(
                    ps[:], stat, rsrc[:, base : base + CAND], start=True, stop=True
                )
                nc.scalar.copy(rowbuf[:], ps[:])
                ov = outv_sb[r // OCHUNK]
                oi = outi_sb[r // OCHUNK]
                ro = r % OCHUNK
                vs = ov[:, ro * 8 : (ro + 1) * 8]
                nc.vector.max(out=vs, in_=rowbuf[:])
                nc.vector.max_index(
                    out=oi[:, ro * 8 : (ro + 1) * 8], in_max=vs, in_values=rowbuf[:]
                )
                if ro == OCHUNK - 1:
                    ci = r // OCHUNK
                    w = OCHUNK * 8
                    nc.sync.dma_start(
                        outv.ap()[:, ci * w : (ci + 1) * w], ov[:]
                    )
                    nc.sync.dma_start(
                        outi.ap()[:, ci * w : (ci + 1) * w], oi[:]
                    )
    nc.compile()
    return nc


def _split3(v):
    """fp32 -> three bf16 arrays whose sum reproduces v to ~2^-27 rel."""
    v = np.asarray(v, np.float32)
    h = v.astype(BF16)
    r = v - h.astype(np.float32)
    m = r.astype(BF16)
    l = (r - m.astype(np.float32)).astype(BF16)
    return h, m, l


def _slab_blocks(pts):
    """4x4x4 quantile partition -> perm [N] s.t. block r = perm[128r:128r+128]."""
    ix = np.argsort(pts[:, 0], kind="stable")
    out = []
    for i in range(4):
        sx = ix[i * 2048 : (i + 1) * 2048]
        iy = sx[np.argsort(pts[sx, 1], kind="stable")]
        for j in range(4):
            sy = iy[j * 512 : (j + 1) * 512]
            iz = sy[np.argsort(pts[sy, 2], kind="stable")]
            out.append(iz)
    return np.concatenate(out)


# per-coordinate split-product row schedule: (query component, cand component)
_ROWS = ((0, 0), (0, 1), (1, 0), (0, 2), (2, 0), (1, 1))


class _Job:
    """Host-side bucketization state for one (queries, candidates) job."""

    def __init__(self, q, c):
        self.q, self.c = q, c
        self.perm = _slab_blocks(q)
        qs = q[self.perm]  # sorted queries, block r = rows 128r:128r+128
        self.qs = qs
        c64 = c.astype(np.float64)

        lo = np.empty((NB, 3)); hi = np.empty((NB, 3)); marg = np.full(NB, MARGIN)
        cand_map = np.zeros((NB, CAND), np.int64)
        counts = np.zeros(NB, np.int64)
        gath = np.zeros((NB, CAND, 3), np.float32)
        for r in range(NB):
            p = qs[r * P : (r + 1) * P].astype(np.float64)
            lo[r], hi[r] = p.min(0), p.max(0)
            m = MARGIN
            for _ in range(30):
                sel = np.nonzero(
                    np.all((c64 >= lo[r] - m) & (c64 <= hi[r] + m), axis=1)
                )[0]
                if len(sel) <= CAND:
                    break
                m *= 0.85
            marg[r] = m
            k = len(sel)
            counts[r] = k
            cand_map[r, :k] = sel
            if k < CAND:
                cand_map[r, k:] = sel[0] if k else 0
            gath[r] = c[cand_map[r]]
        self.lo, self.hi, self.marg = lo, hi, marg
        self.cand_map, self.counts = cand_map, counts

        # lhsT [K, N] from sorted queries; rhs [K, NB*CAND] from gathered cands
        lhsT = np.zeros((K, N), BF16)
        rhs = np.zeros((K, NB * CAND), BF16)
        g = gath.reshape(NB * CAND, 3)
        row = 0
        for k in range(3):
            a = _split3(2.0 * qs[:, k])
            b = _split3(g[:, k])
            for ai, bi in _ROWS:
                lhsT[row] = a[ai]
                rhs[row] = b[bi]
                row += 1
        a = _split3(-np.sum(qs * qs, axis=1))
        for t in range(3):
            lhsT[row] = a[t]
            rhs[row] = np.ones(NB * CAND, BF16)
            row += 1
        b = _split3(np.sum(g * g, axis=1))
        for t in range(3):
            lhsT[row] = np.full(N, -1.0, BF16)
            rhs[row] = b[t]
            row += 1
        assert row == K
        self.in_map = {"lhsT": lhsT, "rhs": rhs}

    def finish(self, res_map):
        """Decode device outputs; exact host fallback where the coverage
        guarantee fails. Returns (dist [N], idx [N]) in original order."""
        vals = res_map["outv"][:, ::8].T.reshape(-1)  # sorted-query order
        slots = res_map["outi"][:, ::8].T.reshape(-1).astype(np.int64)
        d_dev = -vals.astype(np.float64)
        blk = np.arange(N) // P
        idx_dev = self.cand_map[blk, slots]

        qs64 = self.qs.astype(np.float64)
        r_in = np.minimum(
            (qs64 - self.lo[blk]).min(1), (self.hi[blk] - qs64).min(1)
        )
        m_q = self.marg[blk] + np.maximum(r_in, 0.0)
        ok = np.sqrt(np.maximum(d_dev, 0.0)) + 1e-3 <= m_q
        ok &= self.counts[blk] > 0
        bad = np.nonzero(~ok)[0]
        if len(bad):
            qb = self.qs[bad]
            d = (
                np.sum(qb * qb, axis=1, keepdims=True)
                - 2.0 * (qb @ self.c.T)
                + np.sum(self.c * self.c, axis=1)[None, :]
            )
            idx_dev[bad] = np.argmin(d, axis=1)
            d_dev[bad] = d[np.arange(len(bad)), idx_dev[bad]]

        dist = np.empty(N); idx = np.empty(N, np.int64)
        dist[self.perm] = d_dev
        idx[self.perm] = idx_dev
        return dist, idx


def kernel(x, y):
    global LAST_RESULTS
    x = np.ascontiguousarray(x, dtype=np.float32)
    y = np.ascontiguousarray(y, dtype=np.float32)

    jobs = []
    for b in range(B):
        jobs.append(_Job(x[b], y[b]))
        jobs.append(_Job(y[b], x[b]))

    if "nc" not in _CACHE:
        _CACHE["nc"] = _build()
    res = run_bass_kernel_spmd(
        _CACHE["nc"],
        [j.in_map for j in jobs],
        core_ids=list(range(8)),
        trace=TRACE,
        **TRACE_KW,
    )
    LAST_RESULTS = res

    total = 0.0
    for b in range(B):
        dist1, idx1 = jobs[2 * b].finish(res.results[2 * b])
        dist2, idx2 = jobs[2 * b + 1].finish(res.results[2 * b + 1])
        count1 = np.bincount(idx1, minlength=N).astype(np.float64)
        count2 = np.bincount(idx2, minlength=N).astype(np.float64)
        w1 = 1.0 / (count1[idx1] + EPS)
        w2 = 1.0 / (count2[idx2] + EPS)
        loss1 = np.mean(1.0 - np.exp(-dist1 * ALPHA) * w1)
        loss2 = np.mean(1.0 - np.exp(-dist2 * ALPHA) * w2)
        total += (loss1 + loss2) / 2.0
    return np.array(total / B, dtype=np.float32)


# revision 14
# speedup vs baseline: 22.2923x; 1.2531x over previous
"""Density-aware Chamfer distance kernel for Trainium2 (8 NeuronCores).

Problem: x,y [4, 8192, 3] f32. Needs, per batch: row-min + argmin of the
8192x8192 pairwise squared-distance matrix in both directions, density
counts, then a scalar loss.

Strategy (SPMD, 8 cores = 4 batches x 2 directions); each core runs one
"queries vs candidates" nearest-neighbor job:
  core 2b  : queries=x[b], candidates=y[b]  -> dist1/idx1
  core 2b+1: queries=y[b], candidates=x[b]  -> dist2/idx2

Instead of scanning all 8192 candidates per query (memory/vector-bound),
the host groups queries into 64 spatially-compact blocks of 128 (4x4x4
quantile slabs) and gathers, per block, the candidates inside the block
bbox expanded by MARGIN. A query whose device-found nearest distance
exceeds its guaranteed-coverage radius is recomputed exactly on host
(rare: ~0.4%); correctness never depends on the heuristic.

Device per block: PE computes s = 2*q.c - qq - cc = -(squared distance)
with a K=24 split-bf16 matmul (fp32-accurate, 4x faster than fp32
matmul), ScalarE copies PSUM->SBUF, VectorE max/max_index reduce to the
top-1 value+index per query (max of negated distance = min distance;
first-index tie-break + ascending-gathered candidates match jnp.argmin).
The O(N) tail (bincount, weights, loss) runs on host.
"""

import ml_dtypes
import numpy as np

import concourse.bacc as bacc
import concourse.mybir as mybir
import concourse.tile as tile
from concourse.bass_utils import run_bass_kernel_spmd

BF16 = ml_dtypes.bfloat16

B = 4
N = 8192  # points per cloud
P = 128  # partitions = queries per block
NB = N // P  # 64 blocks
CAND = 384  # candidate slots per block
RCHUNK = 8  # rhs DMA streaming: blocks per chunk
OCHUNK = 16  # output DMA: blocks per chunk
K = 24  # contraction rows of the split-bf16 distance matmul
MARGIN = 0.0625
ALPHA = 1000.0
EPS = 1e-6

TRACE = False
TRACE_KW = {}
LAST_RESULTS = None  # BassKernelResults of the most recent run (for test.py)

_CACHE = {}


def _build():
    nc = bacc.Bacc("TRN2", target_bir_lowering=False)
    f32 = mybir.dt.float32
    bf16 = mybir.dt.bfloat16
    lhsT = nc.dram_tensor("lhsT", [K, N], bf16, kind="ExternalInput")
    rhs = nc.dram_tensor("rhs", [K, NB * CAND], bf16, kind="ExternalInput")
    outv = nc.dram_tensor("outv", [P, NB * 8], f32, kind="ExternalOutput")
    outi = nc.dram_tensor("outi", [P, NB * 8], mybir.dt.uint32, kind="ExternalOutput")

    with tile.TileContext(nc) as tc:
        with (
            tc.tile_pool(name="const", bufs=1) as cpool,
            tc.tile_pool(name="rowbuf", bufs=4) as rpool,
            tc.tile_pool(name="psum", bufs=6, space="PSUM") as ppool,
        ):
            # stream inputs in chunks (separate tiles so matmuls only wait
            # on the chunk they read); block 0's data is triggered first so
            # compute starts as early as possible
            lhsT_a = cpool.tile([K, P * RCHUNK], bf16, name="lhsT_a")
            nc.sync.dma_start(lhsT_a[:], lhsT.ap()[:, : P * RCHUNK])
            rhs_sb = []
            w = RCHUNK * CAND
            t0 = cpool.tile([K, w], bf16, name="rhs0")
            nc.sync.dma_start(t0[:], rhs.ap()[:, :w])
            rhs_sb.append(t0)
            lhsT_b = cpool.tile([K, N - P * RCHUNK], bf16, name="lhsT_b")
            nc.sync.dma_start(lhsT_b[:], lhsT.ap()[:, P * RCHUNK :])
            for ci in range(1, NB // RCHUNK):
                t = cpool.tile([K, w], bf16, name=f"rhs{ci}")
                nc.sync.dma_start(t[:], rhs.ap()[:, ci * w : (ci + 1) * w])
                rhs_sb.append(t)

            def stat_slice(r):
                if r < RCHUNK:
                    return lhsT_a[:, r * P : (r + 1) * P]
                rr = r - RCHUNK
                return lhsT_b[:, rr * P : (rr + 1) * P]
            outv_sb = [
                cpool.tile([P, OCHUNK * 8], f32, name=f"ov{ci}")
                for ci in range(NB // OCHUNK)
            ]
            outi_sb = [
                cpool.tile([P, OCHUNK * 8], mybir.dt.uint32, name=f"oi{ci}")
                for ci in range(NB // OCHUNK)
            ]

            for r in range(NB):
                rowbuf = rpool.tile([P, CAND], f32)
                stat = stat_slice(r)
                ps = ppool.tile([P, CAND], f32)
                rsrc = rhs_sb[r // RCHUNK]
                base = (r % RCHUNK) * CAND
                nc.tensor.matmul(
                    ps[:], stat, rsrc[:, base : base + CAND], start=True, stop=True
                )
                nc.scalar.copy(rowbuf[:], ps[:])
                ov = outv_sb[r // OCHUNK]
                oi = outi_sb[r // OCHUNK]
                ro = r % OCHUNK
                vs = ov[:, ro * 8 : (ro + 1) * 8]
                nc.vector.max(out=vs, in_=rowbuf[:])
                nc.vector.max_index(
                    out=oi[:, ro * 8 : (ro + 1) * 8], in_max=vs, in_values=rowbuf[:]
                )
                if ro == OCHUNK - 1:
                    ci = r // OCHUNK
                    w = OCHUNK * 8
                    nc.sync.dma_start(
                        outv.ap()[:, ci * w : (ci + 1) * w], ov[:]
                    )
                    nc.sync.dma_start(
                        outi.ap()[:, ci * w : (ci + 1) * w], oi[:]
                    )
    nc.compile()
    return nc


def _split3(v):
    """fp32 -> three bf16 arrays whose sum reproduces v to ~2^-27 rel."""
    v = np.asarray(v, np.float32)
    h = v.astype(BF16)
    r = v - h.astype(np.float32)
    m = r.astype(BF16)
    l = (r - m.astype(np.float32)).astype(BF16)
    return h, m, l


def _slab_blocks(pts):
    """4x4x4 quantile partition -> perm [N] s.t. block r = perm[128r:128r+128]."""
    ix = np.argsort(pts[:, 0], kind="stable")
    out = []
    for i in range(4):
        sx = ix[i * 2048 : (i + 1) * 2048]
        iy = sx[np.argsort(pts[sx, 1], kind="stable")]
        for j in range(4):
            sy = iy[j * 512 : (j + 1) * 512]
            iz = sy[np.argsort(pts[sy, 2], kind="stable")]
            out.append(iz)
    return np.concatenate(out)


# per-coordinate split-product row schedule: (query component, cand component)
_ROWS = ((0, 0), (0, 1), (1, 0), (0, 2), (2, 0), (1, 1))


class _Job:
    """Host-side bucketization state for one (queries, candidates) job."""

    def __init__(self, q, c):
        self.q, self.c = q, c
        self.perm = _slab_blocks(q)
        qs = q[self.perm]  # sorted queries, block r = rows 128r:128r+128
        self.qs = qs
        c64 = c.astype(np.float64)

        lo = np.empty((NB, 3)); hi = np.empty((NB, 3)); marg = np.full(NB, MARGIN)
        cand_map = np.zeros((NB, CAND), np.int64)
        counts = np.zeros(NB, np.int64)
        gath = np.zeros((NB, CAND, 3), np.float32)
        for r in range(NB):
            p = qs[r * P : (r + 1) * P].astype(np.float64)
            lo[r], hi[r] = p.min(0), p.max(0)
            m = MARGIN
            for _ in range(30):
                sel = np.nonzero(
                    np.all((c64 >= lo[r] - m) & (c64 <= hi[r] + m), axis=1)
                )[0]
                if len(sel) <= CAND:
                    break
                m *= 0.85
            marg[r] = m
            k = len(sel)
            counts[r] = k
            cand_map[r, :k] = sel
            if k < CAND:
                cand_map[r, k:] = sel[0] if k else 0
            gath[r] = c[cand_map[r]]
        self.lo, self.hi, self.marg = lo, hi, marg
        self.cand_map, self.counts = cand_map, counts

        # lhsT [K, N] from sorted queries; rhs [K, NB*CAND] from gathered cands
        lhsT = np.zeros((K, N), BF16)
        rhs = np.zeros((K, NB * CAND), BF16)
        g = gath.reshape(NB * CAND, 3)
        row = 0
        for k in range(3):
            a = _split3(2.0 * qs[:, k])
            b = _split3(g[:, k])
            for ai, bi in _ROWS:
                lhsT[row] = a[ai]
                rhs[row] = b[bi]
                row += 1
        a = _split3(-np.sum(qs * qs, axis=1))
        for t in range(3):
            lhsT[row] = a[t]
            rhs[row] = np.ones(NB * CAND, BF16)
            row += 1
        b = _split3(np.sum(g * g, axis=1))
        for t in range(3):
            lhsT[row] = np.full(N, -1.0, BF16)
            rhs[row] = b[t]
            row += 1
        assert row == K
        self.in_map = {"lhsT": lhsT, "rhs": rhs}

    def finish(self, res_map):
        """Decode device outputs; exact host fallback where the coverage
        guarantee fails. Returns (dist [N], idx [N]) in original order."""
        vals = res_map["outv"][:, ::8].T.reshape(-1)  # sorted-query order
        slots = res_map["outi"][:, ::8].T.reshape(-1).astype(np.int64)
        d_dev = -vals.astype(np.float64)
        blk = np.arange(N) // P
        idx_dev = self.cand_map[blk, slots]

        qs64 = self.qs.astype(np.float64)
        r_in = np.minimum(
            (qs64 - self.lo[blk]).min(1), (self.hi[blk] - qs64).min(1)
        )
        m_q = self.marg[blk] + np.maximum(r_in, 0.0)
        ok = np.sqrt(np.maximum(d_dev, 0.0)) + 1e-3 <= m_q
        ok &= self.counts[blk] > 0
        bad = np.nonzero(~ok)[0]
        if len(bad):
            qb = self.qs[bad]
            d = (
                np.sum(qb * qb, axis=1, keepdims=True)
                - 2.0 * (qb @ self.c.T)
                + np.sum(self.c * self.c, axis=1)[None, :]
            )
            idx_dev[bad] = np.argmin(d, axis=1)
            d_dev[bad] = d[np.arange(len(bad)), idx_dev[bad]]

        dist = np.empty(N); idx = np.empty(N, np.int64)
        dist[self.perm] = d_dev
        idx[self.perm] = idx_dev
        return dist, idx


def kernel(x, y):
    global LAST_RESULTS
    x = np.ascontiguousarray(x, dtype=np.float32)
    y = np.ascontiguousarray(y, dtype=np.float32)

    jobs = []
    for b in range(B):
        jobs.append(_Job(x[b], y[b]))
        jobs.append(_Job(y[b], x[b]))

    if "nc" not in _CACHE:
        _CACHE["nc"] = _build()
    res = run_bass_kernel_spmd(
        _CACHE["nc"],
        [j.in_map for j in jobs],
        core_ids=list(range(8)),
        trace=TRACE,
        **TRACE_KW,
    )
    LAST_RESULTS = res

    total = 0.0
    for b in range(B):
        dist1, idx1 = jobs[2 * b].finish(res.results[2 * b])
        dist2, idx2 = jobs[2 * b + 1].finish(res.results[2 * b + 1])
        count1 = np.bincount(idx1, minlength=N).astype(np.float64)
        count2 = np.bincount(idx2, minlength=N).astype(np.float64)
        w1 = 1.0 / (count1[idx1] + EPS)
        w2 = 1.0 / (count2[idx2] + EPS)
        loss1 = np.mean(1.0 - np.exp(-dist1 * ALPHA) * w1)
        loss2 = np.mean(1.0 - np.exp(-dist2 * ALPHA) * w2)
        total += (loss1 + loss2) / 2.0
    return np.array(total / B, dtype=np.float32)


# revision 18
# speedup vs baseline: 24.5008x; 1.0991x over previous
"""Density-aware Chamfer distance kernel for Trainium2 (8 NeuronCores).

Problem: x,y [4, 8192, 3] f32. Needs, per batch: row-min + argmin of the
8192x8192 pairwise squared-distance matrix in both directions, density
counts, then a scalar loss.

Strategy (SPMD, 8 cores = 4 batches x 2 directions); each core runs one
"queries vs candidates" nearest-neighbor job:
  core 2b  : queries=x[b], candidates=y[b]  -> dist1/idx1
  core 2b+1: queries=y[b], candidates=x[b]  -> dist2/idx2

Instead of scanning all 8192 candidates per query (memory/vector-bound),
the host groups queries into 64 spatially-compact blocks of 128 (4x4x4
quantile slabs) and gathers, per block, the candidates inside the block
bbox expanded by MARGIN. A query whose device-found nearest distance
exceeds its guaranteed-coverage radius is recomputed exactly on host
(rare: ~0.4%); correctness never depends on the heuristic.

Device per block: PE computes s = 2*q.c - qq - cc = -(squared distance)
with a K=24 split-bf16 matmul (fp32-accurate, 4x faster than fp32
matmul), ScalarE copies PSUM->SBUF, VectorE max/max_index reduce to the
top-1 value+index per query (max of negated distance = min distance;
first-index tie-break + ascending-gathered candidates match jnp.argmin).
The O(N) tail (bincount, weights, loss) runs on host.
"""

import ml_dtypes
import numpy as np

import concourse.bacc as bacc
import concourse.mybir as mybir
import concourse.tile as tile
from concourse.bass_utils import run_bass_kernel_spmd

BF16 = ml_dtypes.bfloat16

B = 4
N = 8192  # points per cloud
P = 128  # partitions = queries per block
NB = N // P  # 64 blocks
CAND = 384  # candidate slots per block
GRP = 8  # blocks per DVE reduce/find_index group (find_index8 wants 8)
RCHUNK = GRP  # rhs DMA streaming: blocks per chunk
K = 24  # contraction rows of the split-bf16 distance matmul
MARGIN = 0.0625
ALPHA = 1000.0
EPS = 1e-6

TRACE = False
TRACE_KW = {}
LAST_RESULTS = None  # BassKernelResults of the most recent run (for test.py)

_CACHE = {}


def _build():
    nc = bacc.Bacc("TRN2", target_bir_lowering=False)
    f32 = mybir.dt.float32
    bf16 = mybir.dt.bfloat16
    lhsT = nc.dram_tensor("lhsT", [K, N], bf16, kind="ExternalInput")
    rhs = nc.dram_tensor("rhs", [K, NB * CAND], bf16, kind="ExternalInput")
    outv = nc.dram_tensor("outv", [P, NB], f32, kind="ExternalOutput")
    outi = nc.dram_tensor("outi", [P, NB], mybir.dt.uint32, kind="ExternalOutput")

    with tile.TileContext(nc) as tc:
        with (
            tc.tile_pool(name="const", bufs=1) as cpool,
            tc.tile_pool(name="rowbuf", bufs=4) as rpool,
            tc.tile_pool(name="psum", bufs=6, space="PSUM") as ppool,
        ):
            # stream inputs in chunks (separate tiles so matmuls only wait
            # on the chunk they read); block 0's data is triggered first so
            # compute starts as early as possible
            lhsT_a = cpool.tile([K, P * RCHUNK], bf16, name="lhsT_a")
            nc.sync.dma_start(lhsT_a[:], lhsT.ap()[:, : P * RCHUNK])
            rhs_sb = []
            w = RCHUNK * CAND
            t0 = cpool.tile([K, w], bf16, name="rhs0")
            nc.sync.dma_start(t0[:], rhs.ap()[:, :w])
            rhs_sb.append(t0)
            lhsT_b = cpool.tile([K, N - P * RCHUNK], bf16, name="lhsT_b")
            nc.sync.dma_start(lhsT_b[:], lhsT.ap()[:, P * RCHUNK :])
            for ci in range(1, NB // RCHUNK):
                t = cpool.tile([K, w], bf16, name=f"rhs{ci}")
                nc.sync.dma_start(t[:], rhs.ap()[:, ci * w : (ci + 1) * w])
                rhs_sb.append(t)

            def stat_slice(r):
                if r < RCHUNK:
                    return lhsT_a[:, r * P : (r + 1) * P]
                rr = r - RCHUNK
                return lhsT_b[:, rr * P : (rr + 1) * P]
            ngrp = NB // GRP
            outv_sb = [
                cpool.tile([P, NB // 2], f32, name=f"ov{ci}") for ci in range(2)
            ]
            outi_sb = [
                cpool.tile([P, NB // 2], mybir.dt.uint32, name=f"oi{ci}")
                for ci in range(2)
            ]

            for g in range(ngrp):
                grpbuf = rpool.tile([P, GRP * CAND], f32)
                rsrc = rhs_sb[g]
                for b in range(GRP):
                    r = g * GRP + b
                    ps = ppool.tile([P, CAND], f32)
                    nc.tensor.matmul(
                        ps[:],
                        stat_slice(r),
                        rsrc[:, b * CAND : (b + 1) * CAND],
                        start=True,
                        stop=True,
                    )
                    nc.scalar.copy(grpbuf[:, b * CAND : (b + 1) * CAND], ps[:])
                half = g // (ngrp // 2)
                go = (g % (ngrp // 2)) * GRP
                vs = outv_sb[half][:, go : go + GRP]
                nc.vector.reduce_max(
                    out=vs,
                    in_=grpbuf[:].rearrange("p (b c) -> p b c", c=CAND),
                    axis=mybir.AxisListType.X,
                )
                nc.vector.max_index(
                    out=outi_sb[half][:, go : go + GRP], in_max=vs, in_values=grpbuf[:]
                )
                if g % (ngrp // 2) == ngrp // 2 - 1:
                    w = NB // 2
                    nc.sync.dma_start(
                        outv.ap()[:, half * w : (half + 1) * w], outv_sb[half][:]
                    )
                    nc.sync.dma_start(
                        outi.ap()[:, half * w : (half + 1) * w], outi_sb[half][:]
                    )
    nc.compile()
    return nc


def _split3(v):
    """fp32 -> three bf16 arrays whose sum reproduces v to ~2^-27 rel."""
    v = np.asarray(v, np.float32)
    h = v.astype(BF16)
    r = v - h.astype(np.float32)
    m = r.astype(BF16)
    l = (r - m.astype(np.float32)).astype(BF16)
    return h, m, l


def _slab_blocks(pts):
    """4x4x4 quantile partition -> perm [N] s.t. block r = perm[128r:128r+128]."""
    ix = np.argsort(pts[:, 0], kind="stable")
    out = []
    for i in range(4):
        sx = ix[i * 2048 : (i + 1) * 2048]
        iy = sx[np.argsort(pts[sx, 1], kind="stable")]
        for j in range(4):
            sy = iy[j * 512 : (j + 1) * 512]
            iz = sy[np.argsort(pts[sy, 2], kind="stable")]
            out.append(iz)
    return np.concatenate(out)


# per-coordinate split-product row schedule: (query component, cand component)
_ROWS = ((0, 0), (0, 1), (1, 0), (0, 2), (2, 0), (1, 1))


class _Job:
    """Host-side bucketization state for one (queries, candidates) job."""

    def __init__(self, q, c):
        self.q, self.c = q, c
        self.perm = _slab_blocks(q)
        qs = q[self.perm]  # sorted queries, block r = rows 128r:128r+128
        self.qs = qs
        c64 = c.astype(np.float64)

        lo = np.empty((NB, 3)); hi = np.empty((NB, 3)); marg = np.full(NB, MARGIN)
        cand_map = np.zeros((NB, CAND), np.int64)
        counts = np.zeros(NB, np.int64)
        gath = np.zeros((NB, CAND, 3), np.float32)
        for r in range(NB):
            p = qs[r * P : (r + 1) * P].astype(np.float64)
            lo[r], hi[r] = p.min(0), p.max(0)
            m = MARGIN
            for _ in range(30):
                sel = np.nonzero(
                    np.all((c64 >= lo[r] - m) & (c64 <= hi[r] + m), axis=1)
                )[0]
                if len(sel) <= CAND:
                    break
                m *= 0.85
            marg[r] = m
            k = len(sel)
            counts[r] = k
            cand_map[r, :k] = sel
            if k < CAND:
                cand_map[r, k:] = sel[0] if k else 0
            gath[r] = c[cand_map[r]]
        self.lo, self.hi, self.marg = lo, hi, marg
        self.cand_map, self.counts = cand_map, counts

        # lhsT [K, N] from sorted queries; rhs [K, NB*CAND] from gathered cands
        lhsT = np.zeros((K, N), BF16)
        rhs = np.zeros((K, NB * CAND), BF16)
        g = gath.reshape(NB * CAND, 3)
        row = 0
        for k in range(3):
            a = _split3(2.0 * qs[:, k])
            b = _split3(g[:, k])
            for ai, bi in _ROWS:
                lhsT[row] = a[ai]
                rhs[row] = b[bi]
                row += 1
        a = _split3(-np.sum(qs * qs, axis=1))
        for t in range(3):
            lhsT[row] = a[t]
            rhs[row] = np.ones(NB * CAND, BF16)
            row += 1
        b = _split3(np.sum(g * g, axis=1))
        for t in range(3):
            lhsT[row] = np.full(N, -1.0, BF16)
            rhs[row] = b[t]
            row += 1
        assert row == K
        self.in_map = {"lhsT": lhsT, "rhs": rhs}

    def finish(self, res_map):
        """Decode device outputs; exact host fallback where the coverage
        guarantee fails. Returns (dist [N], idx [N]) in original order."""
        vals = res_map["outv"].T.reshape(-1)  # sorted-query order
        gpos = res_map["outi"].T.reshape(-1).astype(np.int64)
        d_dev = -vals.astype(np.float64)
        blk = np.arange(N) // P
        # find_index8 scanned the whole 8-block group row; the decoded
        # position must fall in this query's own block (a bit-exact value
        # coincidence in a sibling block is detected -> host fallback)
        in_own = (gpos // CAND) == (blk % GRP)
        slots = np.where(in_own, gpos % CAND, 0)
        idx_dev = self.cand_map[blk, slots]

        qs64 = self.qs.astype(np.float64)
        r_in = np.minimum(
            (qs64 - self.lo[blk]).min(1), (self.hi[blk] - qs64).min(1)
        )
        m_q = self.marg[blk] + np.maximum(r_in, 0.0)
        ok = np.sqrt(np.maximum(d_dev, 0.0)) + 1e-3 <= m_q
        ok &= self.counts[blk] > 0
        ok &= in_own
        bad = np.nonzero(~ok)[0]
        if len(bad):
            qb = self.qs[bad]
            d = (
                np.sum(qb * qb, axis=1, keepdims=True)
                - 2.0 * (qb @ self.c.T)
                + np.sum(self.c * self.c, axis=1)[None, :]
            )
            idx_dev[bad] = np.argmin(d, axis=1)
            d_dev[bad] = d[np.arange(len(bad)), idx_dev[bad]]

        dist = np.empty(N); idx = np.empty(N, np.int64)
        dist[self.perm] = d_dev
        idx[self.perm] = idx_dev
        return dist, idx


def kernel(x, y):
    global LAST_RESULTS
    x = np.ascontiguousarray(x, dtype=np.float32)
    y = np.ascontiguousarray(y, dtype=np.float32)

    jobs = []
    for b in range(B):
        jobs.append(_Job(x[b], y[b]))
        jobs.append(_Job(y[b], x[b]))

    if "nc" not in _CACHE:
        _CACHE["nc"] = _build()
    res = run_bass_kernel_spmd(
        _CACHE["nc"],
        [j.in_map for j in jobs],
        core_ids=list(range(8)),
        trace=TRACE,
        **TRACE_KW,
    )
    LAST_RESULTS = res

    total = 0.0
    for b in range(B):
        dist1, idx1 = jobs[2 * b].finish(res.results[2 * b])
        dist2, idx2 = jobs[2 * b + 1].finish(res.results[2 * b + 1])
        count1 = np.bincount(idx1, minlength=N).astype(np.float64)
        count2 = np.bincount(idx2, minlength=N).astype(np.float64)
        w1 = 1.0 / (count1[idx1] + EPS)
        w2 = 1.0 / (count2[idx2] + EPS)
        loss1 = np.mean(1.0 - np.exp(-dist1 * ALPHA) * w1)
        loss2 = np.mean(1.0 - np.exp(-dist2 * ALPHA) * w2)
        total += (loss1 + loss2) / 2.0
    return np.array(total / B, dtype=np.float32)


# revision 21
# speedup vs baseline: 33.6397x; 1.3730x over previous
"""Density-aware Chamfer distance kernel for Trainium2 (8 NeuronCores).

Problem: x,y [4, 8192, 3] f32. Needs, per batch: row-min + argmin of the
8192x8192 pairwise squared-distance matrix in both directions, density
counts, then a scalar loss.

Strategy (SPMD, 8 cores = 4 batches x 2 directions); each core runs one
"queries vs candidates" nearest-neighbor job:
  core 2b  : queries=x[b], candidates=y[b]  -> dist1/idx1
  core 2b+1: queries=y[b], candidates=x[b]  -> dist2/idx2

Instead of scanning all 8192 candidates per query (memory/vector-bound),
the host groups queries into 64 spatially-compact blocks of 128 (4x4x4
quantile slabs) and gathers, per block, the candidates inside the block
bbox expanded by MARGIN. A query whose device-found nearest distance
exceeds its guaranteed-coverage radius is recomputed exactly on host
(rare: ~0.4%); correctness never depends on the heuristic.

Device per block: PE computes s = 2*q.c - qq - cc = -(squared distance)
with a K=24 split-bf16 matmul (fp32-accurate, 4x faster than fp32
matmul), ScalarE copies PSUM->SBUF, VectorE max/max_index reduce to the
top-1 value+index per query (max of negated distance = min distance;
first-index tie-break + ascending-gathered candidates match jnp.argmin).
The O(N) tail (bincount, weights, loss) runs on host.
"""

import ml_dtypes
import numpy as np

import concourse.bacc as bacc
import concourse.mybir as mybir
import concourse.tile as tile
from concourse.bass_utils import run_bass_kernel_spmd

BF16 = ml_dtypes.bfloat16

B = 4
N = 8192  # points per cloud
P = 128  # partitions = queries per block
NB = N // P  # 64 blocks
CAND = 256  # candidate slots per block
GRP = 8  # blocks per DVE reduce/find_index group (find_index8 wants 8)
RCHUNK = GRP  # rhs DMA streaming: blocks per chunk
BANK = 512  # psum bank width in f32
K = 24  # contraction rows of the split-bf16 distance matmul
MARGIN = 0.0625
ALPHA = 1000.0
EPS = 1e-6

TRACE = False
TRACE_KW = {}
LAST_RESULTS = None  # BassKernelResults of the most recent run (for test.py)

_CACHE = {}


def _build():
    nc = bacc.Bacc("TRN2", target_bir_lowering=False)
    f32 = mybir.dt.float32
    bf16 = mybir.dt.bfloat16
    lhsT = nc.dram_tensor("lhsT", [K, N], bf16, kind="ExternalInput")
    rhs = nc.dram_tensor("rhs", [K, NB * CAND], bf16, kind="ExternalInput")
    outv = nc.dram_tensor("outv", [P, NB], f32, kind="ExternalOutput")
    outi = nc.dram_tensor("outi", [P, NB], mybir.dt.uint32, kind="ExternalOutput")

    with tile.TileContext(nc) as tc:
        with (
            tc.tile_pool(name="const", bufs=1) as cpool,
            tc.tile_pool(name="rowbuf", bufs=4) as rpool,
            tc.tile_pool(name="psum", bufs=2, space="PSUM") as ppool,
        ):
            # stream inputs in chunks (separate tiles so matmuls only wait
            # on the chunk they read); block 0's data is triggered first so
            # compute starts as early as possible
            lhsT_a = cpool.tile([K, P * RCHUNK], bf16, name="lhsT_a")
            nc.sync.dma_start(lhsT_a[:], lhsT.ap()[:, : P * RCHUNK])
            rhs_sb = []
            w = RCHUNK * CAND
            t0 = cpool.tile([K, w], bf16, name="rhs0")
            nc.sync.dma_start(t0[:], rhs.ap()[:, :w])
            rhs_sb.append(t0)
            lhsT_b = cpool.tile([K, N - P * RCHUNK], bf16, name="lhsT_b")
            nc.sync.dma_start(lhsT_b[:], lhsT.ap()[:, P * RCHUNK :])
            for ci in range(1, NB // RCHUNK):
                t = cpool.tile([K, w], bf16, name=f"rhs{ci}")
                nc.sync.dma_start(t[:], rhs.ap()[:, ci * w : (ci + 1) * w])
                rhs_sb.append(t)

            def stat_slice(r):
                if r < RCHUNK:
                    return lhsT_a[:, r * P : (r + 1) * P]
                rr = r - RCHUNK
                return lhsT_b[:, rr * P : (rr + 1) * P]
            ngrp = NB // GRP
            gq = 2  # groups per output-DMA quarter
            outv_sb = [
                cpool.tile([P, gq * GRP], f32, name=f"ov{ci}")
                for ci in range(ngrp // gq)
            ]
            outi_sb = [
                cpool.tile([P, gq * GRP], mybir.dt.uint32, name=f"oi{ci}")
                for ci in range(ngrp // gq)
            ]

            for g in range(ngrp):
                grpbuf = rpool.tile([P, GRP * CAND], f32)
                rsrc = rhs_sb[g]
                # 4 blocks per psum tile (1 bank each, bank-aligned), one
                # strided ACT copy per 4 blocks
                for h in range(2):
                    ps = ppool.tile([P, 4 * BANK], f32)
                    for b4 in range(4):
                        b = h * 4 + b4
                        nc.tensor.matmul(
                            ps[:, b4 * BANK : b4 * BANK + CAND],
                            stat_slice(g * GRP + b),
                            rsrc[:, b * CAND : (b + 1) * CAND],
                            start=True,
                            stop=True,
                        )
                    src = ps[:].rearrange("p (b s) -> p b s", s=BANK)[:, :, 0:CAND]
                    dst = grpbuf[:, h * 4 * CAND : (h + 1) * 4 * CAND].rearrange(
                        "p (b s) -> p b s", s=CAND
                    )
                    nc.scalar.copy(dst, src)
                qi, go = g // gq, (g % gq) * GRP
                vs = outv_sb[qi][:, go : go + GRP]
                if g == 0:
                    # stream the first group: reduce each half as soon as its
                    # copy lands instead of waiting for the whole group
                    for h in range(2):
                        nc.vector.reduce_max(
                            out=vs[:, h * 4 : (h + 1) * 4],
                            in_=grpbuf[
                                :, h * 4 * CAND : (h + 1) * 4 * CAND
                            ].rearrange("p (b c) -> p b c", c=CAND),
                            axis=mybir.AxisListType.X,
                        )
                else:
                    nc.vector.reduce_max(
                        out=vs,
                        in_=grpbuf[:].rearrange("p (b c) -> p b c", c=CAND),
                        axis=mybir.AxisListType.X,
                    )
                nc.vector.max_index(
                    out=outi_sb[qi][:, go : go + GRP], in_max=vs, in_values=grpbuf[:]
                )
                if g % gq == gq - 1:
                    w = gq * GRP
                    nc.sync.dma_start(
                        outv.ap()[:, qi * w : (qi + 1) * w], outv_sb[qi][:]
                    )
                    nc.sync.dma_start(
                        outi.ap()[:, qi * w : (qi + 1) * w], outi_sb[qi][:]
                    )
    nc.compile()
    return nc


def _split3(v):
    """fp32 -> three bf16 arrays whose sum reproduces v to ~2^-27 rel."""
    v = np.asarray(v, np.float32)
    h = v.astype(BF16)
    r = v - h.astype(np.float32)
    m = r.astype(BF16)
    l = (r - m.astype(np.float32)).astype(BF16)
    return h, m, l


def _slab_blocks(pts):
    """4x4x4 quantile partition -> perm [N] s.t. block r = perm[128r:128r+128]."""
    ix = np.argsort(pts[:, 0], kind="stable")
    out = []
    for i in range(4):
        sx = ix[i * 2048 : (i + 1) * 2048]
        iy = sx[np.argsort(pts[sx, 1], kind="stable")]
        for j in range(4):
            sy = iy[j * 512 : (j + 1) * 512]
            iz = sy[np.argsort(pts[sy, 2], kind="stable")]
            out.append(iz)
    return np.concatenate(out)


# per-coordinate split-product row schedule: (query component, cand component)
_ROWS = ((0, 0), (0, 1), (1, 0), (0, 2), (2, 0), (1, 1))


class _Job:
    """Host-side bucketization state for one (queries, candidates) job."""

    def __init__(self, q, c):
        self.q, self.c = q, c
        self.perm = _slab_blocks(q)
        qs = q[self.perm]  # sorted queries, block r = rows 128r:128r+128
        self.qs = qs
        c64 = c.astype(np.float64)

        lo = np.empty((NB, 3)); hi = np.empty((NB, 3)); marg = np.full(NB, MARGIN)
        cand_map = np.zeros((NB, CAND), np.int64)
        counts = np.zeros(NB, np.int64)
        gath = np.zeros((NB, CAND, 3), np.float32)
        for r in range(NB):
            p = qs[r * P : (r + 1) * P].astype(np.float64)
            lo[r], hi[r] = p.min(0), p.max(0)
            m = MARGIN
            for _ in range(30):
                sel = np.nonzero(
                    np.all((c64 >= lo[r] - m) & (c64 <= hi[r] + m), axis=1)
                )[0]
                if len(sel) <= CAND:
                    break
                m *= 0.85
            marg[r] = m
            k = len(sel)
            counts[r] = k
            cand_map[r, :k] = sel
            if k < CAND:
                cand_map[r, k:] = sel[0] if k else 0
            gath[r] = c[cand_map[r]]
        self.lo, self.hi, self.marg = lo, hi, marg
        self.cand_map, self.counts = cand_map, counts

        # lhsT [K, N] from sorted queries; rhs [K, NB*CAND] from gathered cands
        lhsT = np.zeros((K, N), BF16)
        rhs = np.zeros((K, NB * CAND), BF16)
        g = gath.reshape(NB * CAND, 3)
        row = 0
        for k in range(3):
            a = _split3(2.0 * qs[:, k])
            b = _split3(g[:, k])
            for ai, bi in _ROWS:
                lhsT[row] = a[ai]
                rhs[row] = b[bi]
                row += 1
        a = _split3(-np.sum(qs * qs, axis=1))
        for t in range(3):
            lhsT[row] = a[t]
            rhs[row] = np.ones(NB * CAND, BF16)
            row += 1
        b = _split3(np.sum(g * g, axis=1))
        for t in range(3):
            lhsT[row] = np.full(N, -1.0, BF16)
            rhs[row] = b[t]
            row += 1
        assert row == K
        self.in_map = {"lhsT": lhsT, "rhs": rhs}

    def finish(self, res_map):
        """Decode device outputs; exact host fallback where the coverage
        guarantee fails. Returns (dist [N], idx [N]) in original order."""
        vals = res_map["outv"].T.reshape(-1)  # sorted-query order
        gpos = res_map["outi"].T.reshape(-1).astype(np.int64)
        d_dev = -vals.astype(np.float64)
        blk = np.arange(N) // P
        # find_index8 scanned the whole 8-block group row; the decoded
        # position must fall in this query's own block (a bit-exact value
        # coincidence in a sibling block is detected -> host fallback)
        in_own = (gpos // CAND) == (blk % GRP)
        slots = np.where(in_own, gpos % CAND, 0)
        idx_dev = self.cand_map[blk, slots]

        qs64 = self.qs.astype(np.float64)
        r_in = np.minimum(
            (qs64 - self.lo[blk]).min(1), (self.hi[blk] - qs64).min(1)
        )
        m_q = self.marg[blk] + np.maximum(r_in, 0.0)
        ok = np.sqrt(np.maximum(d_dev, 0.0)) + 1e-3 <= m_q
        ok &= self.counts[blk] > 0
        ok &= in_own
        bad = np.nonzero(~ok)[0]
        if len(bad):
            qb = self.qs[bad]
            d = (
                np.sum(qb * qb, axis=1, keepdims=True)
                - 2.0 * (qb @ self.c.T)
                + np.sum(self.c * self.c, axis=1)[None, :]
            )
            idx_dev[bad] = np.argmin(d, axis=1)
            d_dev[bad] = d[np.arange(len(bad)), idx_dev[bad]]

        dist = np.empty(N); idx = np.empty(N, np.int64)
        dist[self.perm] = d_dev
        idx[self.perm] = idx_dev
        return dist, idx


def kernel(x, y):
    global LAST_RESULTS
    x = np.ascontiguousarray(x, dtype=np.float32)
    y = np.ascontiguousarray(y, dtype=np.float32)

    jobs = []
    for b in range(B):
        jobs.append(_Job(x[b], y[b]))
        jobs.append(_Job(y[b], x[b]))

    if "nc" not in _CACHE:
        _CACHE["nc"] = _build()
    res = run_bass_kernel_spmd(
        _CACHE["nc"],
        [j.in_map for j in jobs],
        core_ids=list(range(8)),
        trace=TRACE,
        **TRACE_KW,
    )
    LAST_RESULTS = res

    total = 0.0
    for b in range(B):
        dist1, idx1 = jobs[2 * b].finish(res.results[2 * b])
        dist2, idx2 = jobs[2 * b + 1].finish(res.results[2 * b + 1])
        count1 = np.bincount(idx1, minlength=N).astype(np.float64)
        count2 = np.bincount(idx2, minlength=N).astype(np.float64)
        w1 = 1.0 / (count1[idx1] + EPS)
        w2 = 1.0 / (count2[idx2] + EPS)
        loss1 = np.mean(1.0 - np.exp(-dist1 * ALPHA) * w1)
        loss2 = np.mean(1.0 - np.exp(-dist2 * ALPHA) * w2)
        total += (loss1 + loss2) / 2.0
    return np.array(total / B, dtype=np.float32)


# revision 27
# speedup vs baseline: 41.1463x; 1.2231x over previous
"""Density-aware Chamfer distance kernel for Trainium2 (8 NeuronCores).

Problem: x,y [4, 8192, 3] f32. Needs, per batch: row-min + argmin of the
8192x8192 pairwise squared-distance matrix in both directions, density
counts, then a scalar loss.

Strategy (SPMD, 8 cores = 4 batches x 2 directions); each core runs one
"queries vs candidates" nearest-neighbor job:
  core 2b  : queries=x[b], candidates=y[b]  -> dist1/idx1
  core 2b+1: queries=y[b], candidates=x[b]  -> dist2/idx2

Instead of scanning all 8192 candidates per query (memory/vector-bound),
the host groups queries into 64 spatially-compact blocks of 128 (4x4x4
quantile slabs) and gathers, per block, the candidates inside the block
bbox expanded by MARGIN. A query whose device-found nearest distance
exceeds its guaranteed-coverage radius is recomputed exactly on host
(rare: ~0.4%); correctness never depends on the heuristic.

Device per block: PE computes s = 2*q.c - qq - cc = -(squared distance)
with a K=24 split-bf16 matmul (fp32-accurate, 4x faster than fp32
matmul), ScalarE copies PSUM->SBUF, VectorE max/max_index reduce to the
top-1 value+index per query (max of negated distance = min distance;
first-index tie-break + ascending-gathered candidates match jnp.argmin).
The O(N) tail (bincount, weights, loss) runs on host.
"""

import ml_dtypes
import numpy as np

import concourse.bacc as bacc
import concourse.mybir as mybir
import concourse.tile as tile
from concourse.bass_utils import run_bass_kernel_spmd

BF16 = ml_dtypes.bfloat16

B = 4
N = 8192  # points per cloud
P = 128  # partitions = queries per block
NB = N // P  # 64 blocks
NQ4 = NB // 4  # 16 quads of 4 blocks (PE 32-row-group packing)
CAND = 192  # candidate slots per block
GRP = 8  # blocks per DVE reduce/find_index group (find_index8 wants 8)
RCHUNK = GRP  # rhs DMA streaming: blocks per chunk
BANK = 512  # psum bank width in f32
K = 24  # contraction rows of the split-bf16 distance matmul
MARGIN = 0.0625
ALPHA = 1000.0
EPS = 1e-6

TRACE = False
TRACE_KW = {}
LAST_RESULTS = None  # BassKernelResults of the most recent run (for test.py)

_CACHE = {}


def _build():
    nc = bacc.Bacc("TRN2", target_bir_lowering=False)
    f32 = mybir.dt.float32
    bf16 = mybir.dt.bfloat16
    # packed layouts: block r = 4q+i lives in partitions 32i..32i+K of
    # quad-column q, so 4 blocks run concurrently in PE 32-row groups
    lhsT = nc.dram_tensor("lhsT", [P, NQ4 * P], bf16, kind="ExternalInput")
    rhs = nc.dram_tensor("rhs", [P, NQ4 * CAND], bf16, kind="ExternalInput")
    outv = nc.dram_tensor("outv", [P, NB], f32, kind="ExternalOutput")
    outi = nc.dram_tensor("outi", [P, NB], mybir.dt.uint32, kind="ExternalOutput")

    with tile.TileContext(nc) as tc:
        with (
            tc.tile_pool(name="const", bufs=1) as cpool,
            tc.tile_pool(name="rowbuf", bufs=4) as rpool,
            tc.tile_pool(name="psum", bufs=2, space="PSUM") as ppool,
        ):
            # stream inputs in chunks (separate tiles so matmuls only wait
            # on the chunk they read); block 0's data is triggered first so
            # compute starts as early as possible. One chunk = 2 quads.
            lhsT_a = cpool.tile([P, 2 * P], bf16, name="lhsT_a")
            nc.sync.dma_start(lhsT_a[:], lhsT.ap()[:, : 2 * P])
            rhs_sb = []
            w = 2 * CAND
            t0 = cpool.tile([P, w], bf16, name="rhs0")
            nc.sync.dma_start(t0[:], rhs.ap()[:, :w])
            rhs_sb.append(t0)
            lhsT_b = cpool.tile([P, (NQ4 - 2) * P], bf16, name="lhsT_b")
            nc.sync.dma_start(lhsT_b[:], lhsT.ap()[:, 2 * P :])
            for ci in range(1, NQ4 // 2):
                t = cpool.tile([P, w], bf16, name=f"rhs{ci}")
                nc.sync.dma_start(t[:], rhs.ap()[:, ci * w : (ci + 1) * w])
                rhs_sb.append(t)

            def stat_slice(q, i):
                pr = slice(32 * i, 32 * i + K)
                if q < 2:
                    return lhsT_a[pr, q * P : (q + 1) * P]
                return lhsT_b[pr, (q - 2) * P : (q - 1) * P]
            ngrp = NB // GRP
            gq = 2  # groups per output-DMA quarter
            outv_sb = [
                cpool.tile([P, gq * GRP], f32, name=f"ov{ci}")
                for ci in range(ngrp // gq)
            ]
            outi_sb = [
                cpool.tile([P, gq * GRP], mybir.dt.uint32, name=f"oi{ci}")
                for ci in range(ngrp // gq)
            ]

            for g in range(ngrp):
                grpbuf = rpool.tile([P, GRP * CAND], f32)
                rsrc = rhs_sb[g]
                # one quad (4 blocks) per psum tile: 4 concurrent matmuls in
                # distinct PE 32-row groups, one strided ACT copy per quad
                for h in range(2):
                    q = 2 * g + h
                    ps = ppool.tile([P, 4 * BANK], f32)
                    for i in range(4):
                        nc.tensor.matmul(
                            ps[:, i * BANK : i * BANK + CAND],
                            stat_slice(q, i),
                            rsrc[32 * i : 32 * i + K, h * CAND : (h + 1) * CAND],
                            start=True,
                            stop=True,
                            tile_position=(32 * i, 0),
                        )
                    src = ps[:].rearrange("p (b s) -> p b s", s=BANK)[:, :, 0:CAND]
                    dst = grpbuf[:, h * 4 * CAND : (h + 1) * 4 * CAND].rearrange(
                        "p (b s) -> p b s", s=CAND
                    )
                    nc.scalar.copy(dst, src)
                qi, go = g // gq, (g % gq) * GRP
                vs = outv_sb[qi][:, go : go + GRP]
                if g == 0:
                    # stream the first group: reduce each half as soon as its
                    # copy lands instead of waiting for the whole group
                    for h in range(2):
                        nc.vector.reduce_max(
                            out=vs[:, h * 4 : (h + 1) * 4],
                            in_=grpbuf[
                                :, h * 4 * CAND : (h + 1) * 4 * CAND
                            ].rearrange("p (b c) -> p b c", c=CAND),
                            axis=mybir.AxisListType.X,
                        )
                else:
                    nc.vector.reduce_max(
                        out=vs,
                        in_=grpbuf[:].rearrange("p (b c) -> p b c", c=CAND),
                        axis=mybir.AxisListType.X,
                    )
                nc.vector.max_index(
                    out=outi_sb[qi][:, go : go + GRP], in_max=vs, in_values=grpbuf[:]
                )
                if g % gq == gq - 1:
                    w = gq * GRP
                    nc.sync.dma_start(
                        outv.ap()[:, qi * w : (qi + 1) * w], outv_sb[qi][:]
                    )
                    nc.sync.dma_start(
                        outi.ap()[:, qi * w : (qi + 1) * w], outi_sb[qi][:]
                    )
    nc.compile()
    return nc


def _split3(v):
    """fp32 -> three bf16 arrays whose sum reproduces v to ~2^-27 rel."""
    v = np.asarray(v, np.float32)
    h = v.astype(BF16)
    r = v - h.astype(np.float32)
    m = r.astype(BF16)
    l = (r - m.astype(np.float32)).astype(BF16)
    return h, m, l


def _slab_blocks(pts):
    """4x4x4 quantile partition -> perm [N] s.t. block r = perm[128r:128r+128]."""
    ix = np.argsort(pts[:, 0], kind="stable")
    out = []
    for i in range(4):
        sx = ix[i * 2048 : (i + 1) * 2048]
        iy = sx[np.argsort(pts[sx, 1], kind="stable")]
        for j in range(4):
            sy = iy[j * 512 : (j + 1) * 512]
            iz = sy[np.argsort(pts[sy, 2], kind="stable")]
            out.append(iz)
    return np.concatenate(out)


# per-coordinate split-product row schedule: (query component, cand component)
_ROWS = ((0, 0), (0, 1), (1, 0), (0, 2), (2, 0), (1, 1))


def _pack4(flat, w):
    """[K, NB*w] -> [128, NQ4*w]: block r=4q+i row k -> partition 32i+k,
    quad-column q*w.. (PE 32-row-group packing)."""
    t = flat.reshape(K, NQ4, 4, w)
    out = np.zeros((P, NQ4 * w), flat.dtype)
    o3 = out.reshape(P, NQ4, w)
    for i in range(4):
        o3[32 * i : 32 * i + K] = t[:, :, i, :]
    return out


class _Job:
    """Host-side bucketization state for one (queries, candidates) job."""

    def __init__(self, q, c):
        self.q, self.c = q, c
        self.perm = _slab_blocks(q)
        qs = q[self.perm]  # sorted queries, block r = rows 128r:128r+128
        self.qs = qs
        c64 = c.astype(np.float64)

        lo = np.empty((NB, 3)); hi = np.empty((NB, 3)); marg = np.full(NB, MARGIN)
        cand_map = np.zeros((NB, CAND), np.int64)
        counts = np.zeros(NB, np.int64)
        gath = np.zeros((NB, CAND, 3), np.float32)
        for r in range(NB):
            p = qs[r * P : (r + 1) * P].astype(np.float64)
            lo[r], hi[r] = p.min(0), p.max(0)
            m = MARGIN
            for _ in range(30):
                sel = np.nonzero(
                    np.all((c64 >= lo[r] - m) & (c64 <= hi[r] + m), axis=1)
                )[0]
                if len(sel) <= CAND:
                    break
                m *= 0.85
            marg[r] = m
            k = len(sel)
            counts[r] = k
            cand_map[r, :k] = sel
            if k < CAND:
                cand_map[r, k:] = sel[0] if k else 0
            gath[r] = c[cand_map[r]]
        self.lo, self.hi, self.marg = lo, hi, marg
        self.cand_map, self.counts = cand_map, counts

        # lhsT [K, N] from sorted queries; rhs [K, NB*CAND] from gathered cands
        lhsT = np.zeros((K, N), BF16)
        rhs = np.zeros((K, NB * CAND), BF16)
        g = gath.reshape(NB * CAND, 3)
        row = 0
        for k in range(3):
            a = _split3(2.0 * qs[:, k])
            b = _split3(g[:, k])
            for ai, bi in _ROWS:
                lhsT[row] = a[ai]
                rhs[row] = b[bi]
                row += 1
        a = _split3(-np.sum(qs * qs, axis=1))
        for t in range(3):
            lhsT[row] = a[t]
            rhs[row] = np.ones(NB * CAND, BF16)
            row += 1
        b = _split3(np.sum(g * g, axis=1))
        for t in range(3):
            lhsT[row] = np.full(N, -1.0, BF16)
            rhs[row] = b[t]
            row += 1
        assert row == K
        self.in_map = {"lhsT": _pack4(lhsT, P), "rhs": _pack4(rhs, CAND)}

    def finish(self, res_map):
        """Decode device outputs; exact host fallback where the coverage
        guarantee fails. Returns (dist [N], idx [N]) in original order."""
        vals = res_map["outv"].T.reshape(-1)  # sorted-query order
        gpos = res_map["outi"].T.reshape(-1).astype(np.int64)
        d_dev = -vals.astype(np.float64)
        blk = np.arange(N) // P
        # find_index8 scanned the whole 8-block group row; the decoded
        # position must fall in this query's own block (a bit-exact value
        # coincidence in a sibling block is detected -> host fallback)
        in_own = (gpos // CAND) == (blk % GRP)
        slots = np.where(in_own, gpos % CAND, 0)
        idx_dev = self.cand_map[blk, slots]

        qs64 = self.qs.astype(np.float64)
        r_in = np.minimum(
            (qs64 - self.lo[blk]).min(1), (self.hi[blk] - qs64).min(1)
        )
        m_q = self.marg[blk] + np.maximum(r_in, 0.0)
        ok = np.sqrt(np.maximum(d_dev, 0.0)) + 1e-3 <= m_q
        ok &= self.counts[blk] > 0
        ok &= in_own
        bad = np.nonzero(~ok)[0]
        if len(bad):
            qb = self.qs[bad]
            d = (
                np.sum(qb * qb, axis=1, keepdims=True)
                - 2.0 * (qb @ self.c.T)
                + np.sum(self.c * self.c, axis=1)[None, :]
            )
            idx_dev[bad] = np.argmin(d, axis=1)
            d_dev[bad] = d[np.arange(len(bad)), idx_dev[bad]]

        dist = np.empty(N); idx = np.empty(N, np.int64)
        dist[self.perm] = d_dev
        idx[self.perm] = idx_dev
        return dist, idx


def kernel(x, y):
    global LAST_RESULTS
    x = np.ascontiguousarray(x, dtype=np.float32)
    y = np.ascontiguousarray(y, dtype=np.float32)

    jobs = []
    for b in range(B):
        jobs.append(_Job(x[b], y[b]))
        jobs.append(_Job(y[b], x[b]))

    if "nc" not in _CACHE:
        _CACHE["nc"] = _build()
    res = run_bass_kernel_spmd(
        _CACHE["nc"],
        [j.in_map for j in jobs],
        core_ids=list(range(8)),
        trace=TRACE,
        **TRACE_KW,
    )
    LAST_RESULTS = res

    total = 0.0
    for b in range(B):
        dist1, idx1 = jobs[2 * b].finish(res.results[2 * b])
        dist2, idx2 = jobs[2 * b + 1].finish(res.results[2 * b + 1])
        count1 = np.bincount(idx1, minlength=N).astype(np.float64)
        count2 = np.bincount(idx2, minlength=N).astype(np.float64)
        w1 = 1.0 / (count1[idx1] + EPS)
        w2 = 1.0 / (count2[idx2] + EPS)
        loss1 = np.mean(1.0 - np.exp(-dist1 * ALPHA) * w1)
        loss2 = np.mean(1.0 - np.exp(-dist2 * ALPHA) * w2)
        total += (loss1 + loss2) / 2.0
    return np.array(total / B, dtype=np.float32)
